# revision 29
# baseline (speedup 1.0000x reference)
"""CrossAttention kernel for Trainium2, 8 NeuronCores, batch-parallel.

Problem (hardcoded): B=16, S=4096, D=1024; K=77, DE=768; H=16, Dh=64.
  q = hs @ Wq; k = ehs @ Wk; v = ehs @ Wv   (per-head attention, softmax over 77)
  out = concat_heads(softmax(q k^T / 8) v) @ Wo + bo

Sharding: data-parallel over batch - core c gets batches [2c, 2c+1]. No collectives.

Per-core dataflow (all matmuls bf16 = full PE rate; fp32 PSUM accumulate):
  - host supplies hsT [D, S] and ehsT [DE, K] pre-transposed in bf16 so every
    GEMM contracts on partitions with zero on-device transposes.
  - setup: KT[m] = Wk.T @ ehsT per 128-row inner block; V_ext = [V_h | 1] per
    head in natural [K, h*65] layout (ones column yields softmax sums).
  - per 512-col s-tile: QT = Wq.T @ hsT; per head-pair scoresT = KT_h.T @ QT_h
    into one 2-bank PSUM tile, a single Exp activation covers both heads,
    o_ext = V_ext^T @ exp gives attn numerator + sums, reciprocal + gpsimd
    partition-broadcast + DVE multiply normalizes into attT bf16,
  - out[s,d] = attT.T @ Wo + bo in natural row layout -> contiguous DMA out.
"""

import numpy as np
import ml_dtypes

import bass_rust as _bass_rust
import concourse.bacc as bacc
import concourse.mybir as mybir
from concourse.hw_specs import get_activation_tables
from concourse.tile import TileContext
from concourse.bass_utils import run_bass_kernel_spmd


class _OneActSetBacc(bacc.Bacc):
    """Bacc that pins Exp+Ln to the combined natural_log_exp_and_others
    table set. The default chooser alternates between exp_and_others and
    the ln set, paying a ~1.3us ACT_TABLE_LOAD per activation; stripping
    Exp/Ln from every other set (order preserved, so act_func_set_id
    indices stay valid) forces one load for the whole kernel."""

    def insert_act_table_loads(self):
        has_activation = any(
            isinstance(i, mybir.InstActivation)
            for b in self.main_func.blocks
            for i in b.instructions
        )
        if not has_activation:
            return
        keep = {
            mybir.ActivationFunctionType.Exp,
            mybir.ActivationFunctionType.Ln,
        }
        tables = []
        for name, fns in get_activation_tables(self.m.arch).items():
            if name != "natural_log_exp_and_others":
                fns = {f for f in fns if f not in keep}
            tables.append((name, fns))
        _bass_rust.insert_act_table_loads(self, tables)

# Problem constants
B, S, D = 16, 4096, 1024
KJ, DE = 77, 768
H, DH = 16, 64
INNER = H * DH  # 1024
NCORES = 8
BPC = B // NCORES  # batches per core = 2
ST = 512  # s-tile (columns of transposed activations)
NST = BPC * S // ST  # 16 s-tiles per core

F32 = mybir.dt.float32
BF16 = mybir.dt.bfloat16
BF16NP = ml_dtypes.bfloat16

_CACHE = {}


def build_bass():
    nc = _OneActSetBacc()

    hst_d = nc.dram_tensor("hst", [BPC, D, S], BF16, kind="ExternalInput")
    ehst_d = nc.dram_tensor("ehst", [BPC, DE, KJ], BF16, kind="ExternalInput")
    wq_d = nc.dram_tensor("wq", [D, INNER], BF16, kind="ExternalInput")
    wk_d = nc.dram_tensor("wk", [DE, INNER], BF16, kind="ExternalInput")
    wv_d = nc.dram_tensor("wv", [DE, INNER], BF16, kind="ExternalInput")
    wo_d = nc.dram_tensor("wo", [INNER, D], BF16, kind="ExternalInput")
    bo_d = nc.dram_tensor("bo", [D], F32, kind="ExternalInput")
    out_d = nc.dram_tensor("out", [BPC, S, D], F32, kind="ExternalOutput")

    with TileContext(nc) as tc:
        with (
            tc.tile_pool(name="const", bufs=1) as constp,
            tc.tile_pool(name="wq", bufs=8) as wqp,
            tc.tile_pool(name="wo", bufs=8) as wop,
            tc.tile_pool(name="wkv", bufs=6) as wkvp,
            tc.tile_pool(name="hst", bufs=24) as hstp,
            tc.tile_pool(name="qt", bufs=12) as qtp,
            tc.tile_pool(name="att", bufs=16) as attp,
            tc.tile_pool(name="expp", bufs=6) as expp,
            tc.tile_pool(name="smalls", bufs=4) as smallp,
            tc.tile_pool(name="outp", bufs=6) as outp,
            tc.tile_pool(name="ps_qo", bufs=2, space="PSUM") as ps_qo,
            tc.tile_pool(name="ps_att", bufs=3, space="PSUM") as ps_att,
        ):
            # ---- weights needed for the first tile: Wq only ----
            wq_sb = []
            for k in range(8):
                wqk = wqp.tile([128, INNER], BF16, tag="wq", name=f"wq{k}")
                nc.sync.dma_start(out=wqk, in_=wq_d[k * 128:(k + 1) * 128, :])
                wq_sb.append(wqk)

            # ---- hsT prefetch (2 s-tiles deep) ----
            hsT_tiles = {}

            def emit_hst(t):
                bl = t // (S // ST)
                sl = (t % (S // ST)) * ST
                tiles = []
                for k in range(8):
                    hk = hstp.tile([128, ST], BF16, tag="hst", name=f"hsT{t}_{k}")
                    nc.sync.dma_start(
                        out=hk, in_=hst_d[bl, k * 128:(k + 1) * 128, sl:sl + ST]
                    )
                    tiles.append(hk)
                hsT_tiles[t] = tiles

            emit_hst(0)

            # ---- per-batch setup: ehsT, KT, V_ext ----
            kt_sb = [[None] * 8 for _ in range(BPC)]
            vext_sb = [None] * BPC

            def setup_batch(b):
                ehsT = []
                for k in range(6):
                    et = constp.tile(
                        [128, KJ], BF16, tag=f"ehsT{b}_{k}", name=f"ehsT{b}_{k}"
                    )
                    nc.sync.dma_start(
                        out=et, in_=ehst_d[b, k * 128:(k + 1) * 128, :]
                    )
                    ehsT.append(et)
                wk_sb = []
                for k in range(6):
                    wkk = wkvp.tile([128, INNER], BF16, tag="wkv", name=f"wk{b}_{k}")
                    nc.sync.dma_start(out=wkk, in_=wk_d[k * 128:(k + 1) * 128, :])
                    wk_sb.append(wkk)
                # KT[m] = (Wk block m).T @ ehsT  -> [128 inner, 77]
                for m in range(8):
                    pkt = ps_qo.tile([128, ST], F32, tag="qo", name=f"pkt{b}_{m}")
                    for k in range(6):
                        nc.tensor.matmul(
                            pkt[:, 0:KJ],
                            wk_sb[k][:, m * 128:(m + 1) * 128],
                            ehsT[k][:, 0:KJ],
                            start=(k == 0),
                            stop=(k == 5),
                        )
                    ktm = constp.tile(
                        [128, KJ], BF16, tag=f"kt{b}_{m}", name=f"kt{b}_{m}"
                    )
                    nc.vector.tensor_copy(ktm, pkt[:, 0:KJ])
                    kt_sb[b][m] = ktm
                wv_sb = []
                for k in range(6):
                    wvk = wkvp.tile([128, INNER], BF16, tag="wkv", name=f"wv{b}_{k}")
                    nc.sync.dma_start(out=wvk, in_=wv_d[k * 128:(k + 1) * 128, :])
                    wv_sb.append(wvk)
                # V natural layout [77, inner], interleaved with ones cols
                vext = constp.tile(
                    [KJ, H * (DH + 1)], BF16, tag=f"vext{b}", name=f"vext{b}"
                )
                nc.gpsimd.memset(vext, 1.0)
                for n in range(2):
                    psv = ps_qo.tile([128, ST], F32, tag="qo", name=f"psv{b}_{n}")
                    for k in range(6):
                        nc.tensor.matmul(
                            psv[0:KJ, :],
                            ehsT[k][:, 0:KJ],
                            wv_sb[k][:, n * 512:(n + 1) * 512],
                            start=(k == 0),
                            stop=(k == 5),
                        )
                    for j in range(8):
                        h = n * 8 + j
                        nc.vector.tensor_copy(
                            vext[0:KJ, h * 65:h * 65 + 64],
                            psv[0:KJ, j * 64:(j + 1) * 64],
                        )
                vext_sb[b] = vext

            setup_batch(0)

            # ---- weights for the D-section (first used one tile later) ----
            wo_sb = []
            for k in range(8):
                wok = wop.tile([128, D], BF16, tag="wo", name=f"wo{k}")
                nc.sync.dma_start(out=wok, in_=wo_d[k * 128:(k + 1) * 128, :])
                wo_sb.append(wok)
            bo_row = constp.tile([1, D], F32, tag="bo_row")
            nc.sync.dma_start(out=bo_row, in_=bo_d[:].unsqueeze(0))
            bo_sb = constp.tile([128, D], F32, tag="bo")
            nc.gpsimd.partition_broadcast(bo_sb, bo_row[0:1, :])

            emit_hst(1)

            # ---- main loop over s-tiles ----
            # D-section of tile t-1 is emitted interleaved with tile t's
            # attention pairs so the PE stream has dense fill work while
            # ACT computes exp/ln (software pipelining across s-tiles).
            att_prev = {}
            ot_live = {}

            def emit_d_group(td, g):
                bd = td // (S // ST)
                sd = (td % (S // ST)) * ST
                r, n = g // 2, g % 2
                if n == 0:
                    ot_live[r] = outp.tile(
                        [128, D], F32, tag="ot", name=f"out{td}_{r}"
                    )
                ot = ot_live[r]
                pso2 = ps_qo.tile([128, ST], F32, tag="qo", name=f"pso2{td}_{r}_{n}")
                attd = att_prev[td]
                for k in range(8):
                    nc.tensor.matmul(
                        pso2,
                        attd[k][:, r * 128:(r + 1) * 128],
                        wo_sb[k][:, n * 512:(n + 1) * 512],
                        start=(k == 0),
                        stop=(k == 7),
                    )
                nc.vector.tensor_add(
                    ot[:, n * 512:(n + 1) * 512],
                    pso2,
                    bo_sb[:, n * 512:(n + 1) * 512],
                )
                if n == 1:
                    nc.sync.dma_start(
                        out=out_d[bd, sd + r * 128:sd + (r + 1) * 128, :], in_=ot
                    )
                    del ot_live[r]

            for t in range(NST):
                b = t // (S // ST)
                s0 = (t % (S // ST)) * ST

                if t == 1:
                    setup_batch(1)

                # A: prefetch hsT two s-tiles ahead (t and t+1 already issued)
                if t + 2 < NST:
                    emit_hst(t + 2)
                hsT = hsT_tiles.pop(t)

                # B: QT = Wq.T @ hsT  -> 8 tiles [128, ST]
                qt = []
                for m in range(8):
                    psq = ps_qo.tile([128, ST], F32, tag="qo", name=f"psq{t}_{m}")
                    for k in range(8):
                        nc.tensor.matmul(
                            psq,
                            wq_sb[k][:, m * 128:(m + 1) * 128],
                            hsT[k],
                            start=(k == 0),
                            stop=(k == 7),
                        )
                    qm = qtp.tile([128, ST], BF16, tag="qt", name=f"qt{t}_{m}")
                    nc.vector.tensor_copy(qm, psq)
                    qt.append(qm)

                # C: attention per head-pair -> attT 8 tiles [128, ST]
                att = [
                    attp.tile([128, ST], BF16, tag="att", name=f"att{t}_{m}")
                    for m in range(8)
                ]
                for g in range(8):
                    # heads 2g (rows 0:64) and 2g+1 (rows 64:128) of block g
                    pss = ps_att.tile([KJ, 2 * ST], F32, tag="aps", name=f"pss{t}_{g}")
                    for half in range(2):
                        prow = slice(half * 64, half * 64 + 64)
                        nc.tensor.matmul(
                            pss[0:KJ, half * ST:(half + 1) * ST],
                            kt_sb[b][g][prow, 0:KJ],
                            qt[g][prow, :],
                            start=True,
                            stop=True,
                        )
                    ex = expp.tile([KJ, 2 * ST], BF16, tag="exp", name=f"exp{t}_{g}")
                    nc.scalar.activation(
                        ex[0:KJ, :], pss[0:KJ, :], mybir.ActivationFunctionType.Exp
                    )
                    # attn numerator + softmax sums for both heads share one
                    # 2-bank PSUM tile: [0:64, half] = num, [64:65, half] = den
                    pso = ps_att.tile([KJ, 2 * ST], F32, tag="aps", name=f"pso{t}_{g}")
                    for half in range(2):
                        h = 2 * g + half
                        nc.tensor.matmul(
                            pso[0:65, half * ST:(half + 1) * ST],
                            vext_sb[b][0:KJ, h * 65:(h + 1) * 65],
                            ex[0:KJ, half * ST:(half + 1) * ST],
                            start=True,
                            stop=True,
                        )
                    # 1/den via exp(-ln(den)) on ACT: Ln and Exp share one
                    # table set (natural_log_exp_and_others) so no reloads.
                    lnd = smallp.tile([1, 2 * ST], F32, tag="lnd", name=f"lnd{t}_{g}")
                    nc.scalar.activation(
                        lnd[0:1, :], pso[64:65, :], mybir.ActivationFunctionType.Ln
                    )
                    rec = smallp.tile([1, 2 * ST], F32, tag="rec", name=f"rec{t}_{g}")
                    nc.scalar.activation(
                        rec[0:1, :],
                        lnd[0:1, :],
                        mybir.ActivationFunctionType.Exp,
                        scale=-1.0,
                    )
                    rb = smallp.tile([64, 2 * ST], F32, tag="rb", name=f"rb{t}_{g}")
                    nc.gpsimd.partition_broadcast(rb, rec[0:1, :])
                    for half in range(2):
                        prow = slice(half * 64, half * 64 + 64)
                        nc.vector.tensor_mul(
                            att[g][prow, :],
                            pso[0:64, half * ST:(half + 1) * ST],
                            rb[:, half * ST:(half + 1) * ST],
                        )
                    if t > 0:
                        emit_d_group(t - 1, g)

                att_prev.pop(t - 1, None)
                att_prev[t] = att

            # trailing D-section for the last s-tile
            for g in range(8):
                emit_d_group(NST - 1, g)

    nc.compile()
    return nc


def make_in_maps(hidden_states, encoder_hidden_states, Wq, Wk, Wv, Wo, bo):
    """Host-side prep: cast to bf16, pre-transpose activations, shard by batch."""
    hs = np.asarray(hidden_states, dtype=np.float32)
    ehs = np.asarray(encoder_hidden_states, dtype=np.float32)
    hst = np.ascontiguousarray(
        hs.astype(BF16NP).transpose(0, 2, 1)
    )  # [B, D, S] bf16
    ehst = np.ascontiguousarray(
        ehs.astype(BF16NP).transpose(0, 2, 1)
    )  # [B, DE, KJ] bf16
    wq_b = (np.asarray(Wq, dtype=np.float32) * (1.0 / np.sqrt(DH))).astype(BF16NP)
    wk_b = np.asarray(Wk, dtype=np.float32).astype(BF16NP)
    wv_b = np.asarray(Wv, dtype=np.float32).astype(BF16NP)
    wo_b = np.asarray(Wo, dtype=np.float32).astype(BF16NP)
    bo_f = np.ascontiguousarray(np.asarray(bo, dtype=np.float32))

    in_maps = []
    for c in range(NCORES):
        in_maps.append(
            {
                "hst": np.ascontiguousarray(hst[c * BPC:(c + 1) * BPC]),
                "ehst": np.ascontiguousarray(ehst[c * BPC:(c + 1) * BPC]),
                "wq": wq_b,
                "wk": wk_b,
                "wv": wv_b,
                "wo": wo_b,
                "bo": bo_f,
            }
        )
    return in_maps


def kernel(hidden_states, encoder_hidden_states, Wq, Wk, Wv, Wo, bo, **unused):
    if "nc" not in _CACHE:
        _CACHE["nc"] = build_bass()
    nc = _CACHE["nc"]

    in_maps = make_in_maps(
        hidden_states, encoder_hidden_states, Wq, Wk, Wv, Wo, bo
    )
    res = run_bass_kernel_spmd(nc, in_maps, list(range(NCORES)))
    outs = [res.results[c]["out"] for c in range(NCORES)]
    return np.concatenate(outs, axis=0)
